# revision 31
# baseline (speedup 1.0000x reference)
"""BranchLayer kernel for 8 Trainium2 NeuronCores.

Math: out[b, c] = sum_k x[b, idx[k, c]] * w[k, c], with last-write-wins on
duplicate (idx[k,c], c) pairs — i.e. out = x @ dense where
dense[i, c] = w[k_last, c] for the last k with idx[k, c] == i.

Strategy (sharding_hint): shard the COLS=16384 column dim of dense across the
8 cores (2048 columns each); x is replicated. The host scatters w into dense
(cheap index bookkeeping) and quantizes it to fp8-e3m4 (x stays fp16), so the
dominant HBM stream halves vs fp16 and drops below the TensorE floor of
65536 cycles (~27us @2.4GHz) — the kernel is PE-bound, so the x load is
split into pieces interleaved with the first dense chunks to start the
matmul pipeline (and the PE clock ramp) as early as possible. Output ships
fp16 (values pre-scaled by W_SCALE); the host upcasts and descales.
"""

import numpy as np
import ml_dtypes

import concourse.bass as bass
import concourse.bacc as bacc
import concourse.mybir as mybir
import concourse.tile as tile
from concourse import bass_utils

F16 = np.float16
E3M4 = ml_dtypes.float8_e3m4

# Problem shape (hardcoded per task contract).
N_IN = 4096
N_NPB = 64
N_B = 64
N_NEXT_H = 256
COLS = N_B * N_NEXT_H  # 16384
BATCH = 128
N_CORES = 8

COLS_PER_CORE = COLS // N_CORES  # 2048
N_BLOCK = 512                    # output columns per PSUM block (one bank)
NUM_BLOCKS = COLS_PER_CORE // N_BLOCK  # 4
N_ITILES = N_IN // 128           # 32 contraction tiles

W_SCALE = 64.0                   # dense pre-scale so e3m4 sees ~[-4, 4]
E3M4_MAX = 15.5
X_PIECES = 4                     # x DMA split (1024 cols each)

_CACHE = {}


def _build_program(repeats=1, dbufs=4, chunks=8, rest_chunks=8,
                   warmup=10, x_pieces=X_PIECES, out_eng="scalar"):
    """One SPMD Bass program; all 8 cores run it on different dense shards.

    repeats>1 loops the whole pipeline inside one NEFF — used only for
    repeat-delta HW timing in test.py (tunnel overhead cancels).
    dbufs: dense-tile pool slots (4 = every block's DMA in flight at start).
    chunks: dense DMA chunks for block 0 (fine ⇒ early first matmul);
    rest_chunks: chunks for blocks 1..3 (coarse — HWDGE queue config time
    per dma_start is ~0.6us, so fewer transfers once the pipe is primed).
    warmup: N=256 dummy matmuls on memset-only tiles (no DMA dependency)
    that ramp the PE clock out of its low p-states during the ~2.5us DMA
    start latency — the kernel is PE-bound, so real matmuls must hit full
    clock immediately.
    """
    if repeats > 1 and out_eng == "scalar":
        # In repeat-timing programs a rep's out-DMAs on the scalar HWDGE
        # queue would stall the NEXT rep's load chunks behind a compute
        # wait; route them via SWDGE there. (Irrelevant for the shipped
        # repeats=1 program — nothing follows its outs.)
        out_eng = "gpsimd"
    key = ("nc", repeats, dbufs, chunks, rest_chunks, warmup, x_pieces,
           out_eng)
    if key in _CACHE:
        return _CACHE[key]

    nc = bacc.Bacc(
        "TRN2",
        target_bir_lowering=False,
        debug=False,
        enable_asserts=False,
        num_devices=N_CORES,
    )
    # xT[il, t*128 + b] = x[b, t*128 + il]  (lhsT tiles, fp16)
    xT = nc.dram_tensor("xT", [128, N_IN], mybir.dt.float16, kind="ExternalInput").ap()
    # dns[n, il, t*N_BLOCK + c'] = dense[t*128 + il, n*N_BLOCK + c'] (per-core
    # shard, e3m4, pre-scaled by W_SCALE)
    dns = nc.dram_tensor(
        "dns", [NUM_BLOCKS, 128, N_ITILES * N_BLOCK], mybir.dt.float8e3,
        kind="ExternalInput",
    ).ap()
    out = nc.dram_tensor(
        "out", [BATCH, COLS_PER_CORE], mybir.dt.float16, kind="ExternalOutput"
    ).ap()

    with tile.TileContext(nc) as tc:
        with (
            tc.tile_pool(name="xp", bufs=1) as xp,
            tc.tile_pool(name="dp", bufs=dbufs) as dp,
            tc.tile_pool(name="op", bufs=6) as op,
            tc.tile_pool(name="pp", bufs=2, space="PSUM") as pp,
        ):
            x_sb = xp.tile([128, N_IN], mybir.dt.float16)
            rsize = N_ITILES * N_BLOCK // rest_chunks
            qs = [nc.sync, nc.scalar]
            # Block-0 chunk boundaries (subtile units) grow geometrically:
            # the first matmul only waits for a 2-subtile (128KB) transfer,
            # pulling the whole PE schedule ~0.6us earlier. x pieces are
            # sized so piece p always lands before the chunks that need it.
            if chunks == 8:
                cb = [0, 1, 2, 4, 8, 12, 17, 24, 32]
            else:
                cs = N_ITILES // chunks
                cb = [i * cs for i in range(chunks + 1)]
            if x_pieces == 4:
                xb = [0, 512, 1536, 2560, 4096]
            else:
                xs = N_IN // x_pieces
                xb = [i * xs for i in range(x_pieces + 1)]

            if warmup:
                # PE clock ramp: dummy matmuls against a RAW (non-pool)
                # SBUF tensor — no producer instruction at all, so the tile
                # scheduler sees no deps and the warms issue right after the
                # prologue barrier (~0.7us), while the first loads are still
                # in HWDGE config. Uninitialized operands (even NaN) are
                # harmless: results land in a scratch PSUM bank nobody
                # reads. Small N so the warm queue drains right as the
                # first real chunk lands.
                w_raw = nc.alloc_sbuf_tensor(
                    "warm_src", [128, 256 + 128], mybir.dt.float16
                ).ap()
                warm_ps = pp.tile([BATCH, 256], mybir.dt.float32, tag="warm")
                for _wmm in range(warmup):
                    nc.tensor.matmul(
                        warm_ps[:],
                        w_raw[:, 256:],
                        w_raw[:, :256],
                        start=True,
                        stop=True,
                    )

            for _rep in range(repeats):
                # Block tiles all live at once (dbufs=NUM_BLOCKS) so every
                # load chunk is issued before any compute-dependent
                # instruction lands on the HWDGE queues.
                d_sbs = []
                for _n in range(NUM_BLOCKS):
                    d_sb = dp.tile([128, N_ITILES * N_BLOCK], mybir.dt.float8e3)
                    d_sbs.append(d_sb)
                # The kernel is PE-bound: interleave x pieces with block 0's
                # chunks across the two HWDGE queues so matmul t=0 (needs
                # x piece 0 + d0 chunk 0 only — tile deps are slice-level)
                # starts as soon as the first two transfers land.
                if _rep == 0:
                    # every x piece must fit in the even-h slots below
                    assert chunks >= 2 * x_pieces
                    for h in range(chunks):
                        if h % 2 == 0 and h // 2 < x_pieces:
                            p = h // 2
                            qs[0].dma_start(
                                out=x_sb[:, xb[p]:xb[p + 1]],
                                in_=xT[:, xb[p]:xb[p + 1]],
                            )
                        qs[1 if h % 2 == 0 else 0].dma_start(
                            out=d_sbs[0][:, cb[h] * N_BLOCK:cb[h + 1] * N_BLOCK],
                            in_=dns[0, :, cb[h] * N_BLOCK:cb[h + 1] * N_BLOCK],
                        )
                    # The prologue put ~2.1MB on qs[0] (x + odd chunks) vs
                    # ~0.9MB on qs[1]; give qs[1] the first 4 rest-chunks,
                    # then alternate, so both queues carry ~4.5MB total and
                    # neither straggles into the last block's feed.
                    qsched = [1, 1, 1, 1]
                    rest = range(1, NUM_BLOCKS)
                else:
                    qsched = []
                    rest = range(NUM_BLOCKS)
                qi = 0
                for n in rest:
                    for h in range(rest_chunks):
                        q = qs[qsched[qi] if qi < len(qsched)
                               else (qi + len(qsched)) % 2]
                        q.dma_start(
                            out=d_sbs[n][:, h * rsize:(h + 1) * rsize],
                            in_=dns[n, :, h * rsize:(h + 1) * rsize],
                        )
                        qi += 1
                for n in range(NUM_BLOCKS):
                    d_sb = d_sbs[n]
                    last = n == NUM_BLOCKS - 1
                    if not last:
                        ps = pp.tile([BATCH, N_BLOCK], mybir.dt.float32)
                        for t in range(N_ITILES):
                            nc.tensor.matmul(
                                ps[:],
                                x_sb[:, t * 128:(t + 1) * 128],
                                d_sb[:, t * N_BLOCK:(t + 1) * N_BLOCK],
                                start=(t == 0),
                                stop=(t == N_ITILES - 1),
                            )
                        o_sb = op.tile([BATCH, N_BLOCK], mybir.dt.float16)
                        nc.vector.tensor_copy(out=o_sb[:], in_=ps[:])
                        getattr(nc, out_eng).dma_start(
                            out=out[:, n * N_BLOCK:(n + 1) * N_BLOCK],
                            in_=o_sb[:],
                        )
                        continue
                    # Last block: its drain is the exposed tail, so split
                    # into TWO accumulation groups — group A's stop lands
                    # one matmul before B's, letting A's drain overlap B's
                    # final matmul — and drain in quarters with the out-DMAs
                    # alternating both HWDGE queues so copies and transfers
                    # pipeline.
                    half = N_BLOCK // 2
                    psa = pp.tile([BATCH, half], mybir.dt.float32)
                    psb = pp.tile([BATCH, half], mybir.dt.float32)
                    for t in range(N_ITILES):
                        for g, pst in ((0, psa), (1, psb)):
                            nc.tensor.matmul(
                                pst[:],
                                x_sb[:, t * 128:(t + 1) * 128],
                                d_sb[:, t * N_BLOCK + g * half:
                                     t * N_BLOCK + (g + 1) * half],
                                start=(t == 0),
                                stop=(t == N_ITILES - 1),
                            )
                    psz = half // 2
                    for j, pst in ((0, psa), (1, psa), (2, psb), (3, psb)):
                        o_sb = op.tile([BATCH, psz], mybir.dt.float16)
                        nc.vector.tensor_copy(
                            out=o_sb[:],
                            in_=pst[:, (j % 2) * psz:(j % 2 + 1) * psz],
                        )
                        oq = qs[j % 2] if out_eng == "scalar" else getattr(
                            nc, out_eng)
                        oq.dma_start(
                            out=out[:, n * N_BLOCK + j * psz:
                                    n * N_BLOCK + (j + 1) * psz],
                            in_=o_sb[:],
                        )

    nc.compile()
    aps = {"xT": xT, "dns": dns, "out": out}
    _CACHE[key] = (nc, aps)
    return nc, aps


def _prepare_inputs(x, w, idx):
    x = np.asarray(x, dtype=np.float32)
    w = np.asarray(w, dtype=np.float32)
    idx = np.asarray(idx)

    # Scatter with last-write-wins (ascending k ⇒ later k overwrites earlier,
    # matching torch's index_put / the reference's keep-mask + scatter-add).
    dense = np.zeros((N_IN, COLS), dtype=np.float32)
    cols = np.arange(COLS)
    for k in range(N_NPB):
        dense[idx[k], cols] = w[k]
    dense = np.clip(dense * W_SCALE, -E3M4_MAX, E3M4_MAX)

    # lhsT layout: xT[il, t, b] = x[b, t*128 + il]
    xT = np.ascontiguousarray(
        x.T.reshape(N_ITILES, 128, BATCH).transpose(1, 0, 2).reshape(128, N_IN)
    ).astype(F16)

    in_maps = []
    for core in range(N_CORES):
        dc = dense[:, core * COLS_PER_CORE:(core + 1) * COLS_PER_CORE]
        # D[n, il, t, c'] = dc[t*128 + il, n*N_BLOCK + c']
        D = np.ascontiguousarray(
            dc.reshape(N_ITILES, 128, NUM_BLOCKS, N_BLOCK)
            .transpose(2, 1, 0, 3)
            .reshape(NUM_BLOCKS, 128, N_ITILES * N_BLOCK)
        ).astype(E3M4)
        in_maps.append({"xT": xT, "dns": D})
    return in_maps


def _run(in_maps, trace=False):
    nc, _ = _build_program()
    res = bass_utils.run_bass_kernel_spmd(
        nc, in_maps, core_ids=list(range(N_CORES)), trace=trace
    )
    _CACHE["last_results"] = res
    return res


def _gather(res):
    out = np.concatenate(
        [np.asarray(r["out"], dtype=np.float32) for r in res.results], axis=1
    )
    return (out / W_SCALE).reshape(BATCH, N_B, N_NEXT_H).astype(np.float32)


def kernel(x, w, idx):
    in_maps = _prepare_inputs(x, w, idx)
    out = None
    for attempt in range(3):
        try:
            out = _gather(_run(in_maps, trace=False))
        except Exception:
            # A previously wedged device can fail the first attach; a retry
            # on a fresh execution is usually enough (device resets on
            # attach).
            import time
            time.sleep(2.0)
            continue
        # A wedged device can also return garbage instead of raising —
        # outputs are bounded by ~0.6 here, so NaN/huge values mean the
        # execution itself was bad, not the math.
        if np.isfinite(out).all() and np.abs(out).max() < 100.0:
            return out
    if out is None:
        raise RuntimeError("kernel execution failed repeatedly")
    return out


# revision 32
# speedup vs baseline: 6.1576x; 6.1576x over previous
"""BranchLayer kernel for 8 Trainium2 NeuronCores.

Math: out[b, c] = sum_k x[b, idx[k, c]] * w[k, c], with last-write-wins on
duplicate (idx[k,c], c) pairs — i.e. out = x @ dense where
dense[i, c] = w[k_last, c] for the last k with idx[k, c] == i.

Strategy (sharding_hint): shard the COLS=16384 column dim of dense across the
8 cores (2048 columns each); x is replicated. The host scatters w into dense
(cheap index bookkeeping) and quantizes it to fp8-e3m4 (x stays fp16), so the
dominant HBM stream halves vs fp16 and drops below the TensorE floor of
65536 cycles (~27us @2.4GHz) — the kernel is PE-bound, so the x load is
split into pieces interleaved with the first dense chunks to start the
matmul pipeline (and the PE clock ramp) as early as possible. Output ships
fp16 (values pre-scaled by W_SCALE); the host upcasts and descales.
"""

import numpy as np
import ml_dtypes

import concourse.bass as bass
import concourse.bacc as bacc
import concourse.mybir as mybir
import concourse.tile as tile
from concourse import bass_utils

F16 = np.float16
E3M4 = ml_dtypes.float8_e3m4

# Problem shape (hardcoded per task contract).
N_IN = 4096
N_NPB = 64
N_B = 64
N_NEXT_H = 256
COLS = N_B * N_NEXT_H  # 16384
BATCH = 128
N_CORES = 8

COLS_PER_CORE = COLS // N_CORES  # 2048
N_BLOCK = 512                    # output columns per PSUM block (one bank)
NUM_BLOCKS = COLS_PER_CORE // N_BLOCK  # 4
N_ITILES = N_IN // 128           # 32 contraction tiles

W_SCALE = 64.0                   # dense pre-scale so e3m4 sees ~[-4, 4]
E3M4_MAX = 15.5
X_PIECES = 4                     # x DMA split (1024 cols each)

_CACHE = {}


def _build_program(repeats=1, dbufs=4, chunks=8, rest_chunks=8,
                   warmup=10, x_pieces=X_PIECES, out_eng="scalar"):
    """One SPMD Bass program; all 8 cores run it on different dense shards.

    repeats>1 loops the whole pipeline inside one NEFF — used only for
    repeat-delta HW timing in test.py (tunnel overhead cancels).
    dbufs: dense-tile pool slots (4 = every block's DMA in flight at start).
    chunks: dense DMA chunks for block 0 (fine ⇒ early first matmul);
    rest_chunks: chunks for blocks 1..3 (coarse — HWDGE queue config time
    per dma_start is ~0.6us, so fewer transfers once the pipe is primed).
    warmup: N=256 dummy matmuls on memset-only tiles (no DMA dependency)
    that ramp the PE clock out of its low p-states during the ~2.5us DMA
    start latency — the kernel is PE-bound, so real matmuls must hit full
    clock immediately.
    """
    if repeats > 1 and out_eng == "scalar":
        # In repeat-timing programs a rep's out-DMAs on the scalar HWDGE
        # queue would stall the NEXT rep's load chunks behind a compute
        # wait; route them via SWDGE there. (Irrelevant for the shipped
        # repeats=1 program — nothing follows its outs.)
        out_eng = "gpsimd"
    key = ("nc", repeats, dbufs, chunks, rest_chunks, warmup, x_pieces,
           out_eng)
    if key in _CACHE:
        return _CACHE[key]

    nc = bacc.Bacc(
        "TRN2",
        target_bir_lowering=False,
        debug=False,
        enable_asserts=False,
        num_devices=N_CORES,
    )
    # xT[il, t*128 + b] = x[b, t*128 + il]  (lhsT tiles, fp16)
    xT = nc.dram_tensor("xT", [128, N_IN], mybir.dt.float16, kind="ExternalInput").ap()
    # dns[n, il, t*N_BLOCK + c'] = dense[t*128 + il, n*N_BLOCK + c'] (per-core
    # shard, e3m4, pre-scaled by W_SCALE)
    dns = nc.dram_tensor(
        "dns", [NUM_BLOCKS, 128, N_ITILES * N_BLOCK], mybir.dt.float8e3,
        kind="ExternalInput",
    ).ap()
    out = nc.dram_tensor(
        "out", [BATCH, COLS_PER_CORE], mybir.dt.float16, kind="ExternalOutput"
    ).ap()

    with tile.TileContext(nc) as tc:
        with (
            tc.tile_pool(name="xp", bufs=1) as xp,
            tc.tile_pool(name="dp", bufs=dbufs) as dp,
            tc.tile_pool(name="op", bufs=6) as op,
            tc.tile_pool(name="pp", bufs=2, space="PSUM") as pp,
        ):
            x_sb = xp.tile([128, N_IN], mybir.dt.float16)
            rsize = N_ITILES * N_BLOCK // rest_chunks
            qs = [nc.sync, nc.scalar]
            # Block-0 chunk boundaries (subtile units) grow geometrically:
            # the first matmul only waits for a 2-subtile (128KB) transfer,
            # pulling the whole PE schedule ~0.6us earlier. x pieces are
            # sized so piece p always lands before the chunks that need it.
            if chunks == 8:
                cb = [0, 1, 2, 4, 8, 12, 17, 24, 32]
            else:
                cs = N_ITILES // chunks
                cb = [i * cs for i in range(chunks + 1)]
            if x_pieces == 4:
                # piece 0 covers exactly the subtiles chunk 0-1 need (64KB,
                # same transfer time as chunk 0) — x and dense arrive
                # together for the first matmul.
                xb = [0, 256, 1024, 2560, 4096]
            else:
                xs = N_IN // x_pieces
                xb = [i * xs for i in range(x_pieces + 1)]

            if warmup:
                # PE clock ramp: dummy matmuls against a RAW (non-pool)
                # SBUF tensor — no producer instruction at all, so the tile
                # scheduler sees no deps and the warms issue right after the
                # prologue barrier (~0.7us), while the first loads are still
                # in HWDGE config. Uninitialized operands (even NaN) are
                # harmless: results land in a scratch PSUM bank nobody
                # reads. Small N so the warm queue drains right as the
                # first real chunk lands.
                w_raw = nc.alloc_sbuf_tensor(
                    "warm_src", [128, 256 + 128], mybir.dt.float16
                ).ap()
                warm_ps = pp.tile([BATCH, 256], mybir.dt.float32, tag="warm")
                for _wmm in range(warmup):
                    nc.tensor.matmul(
                        warm_ps[:],
                        w_raw[:, 256:],
                        w_raw[:, :256],
                        start=True,
                        stop=True,
                    )

            for _rep in range(repeats):
                # Block tiles all live at once (dbufs=NUM_BLOCKS) so every
                # load chunk is issued before any compute-dependent
                # instruction lands on the HWDGE queues.
                d_sbs = []
                for _n in range(NUM_BLOCKS):
                    d_sb = dp.tile([128, N_ITILES * N_BLOCK], mybir.dt.float8e3)
                    d_sbs.append(d_sb)
                # The kernel is PE-bound: interleave x pieces with block 0's
                # chunks across the two HWDGE queues so matmul t=0 (needs
                # x piece 0 + d0 chunk 0 only — tile deps are slice-level)
                # starts as soon as the first two transfers land.
                if _rep == 0:
                    # every x piece must fit in the even-h slots below
                    assert chunks >= 2 * x_pieces
                    for h in range(chunks):
                        if h % 2 == 0 and h // 2 < x_pieces:
                            p = h // 2
                            qs[0].dma_start(
                                out=x_sb[:, xb[p]:xb[p + 1]],
                                in_=xT[:, xb[p]:xb[p + 1]],
                            )
                        qs[1 if h % 2 == 0 else 0].dma_start(
                            out=d_sbs[0][:, cb[h] * N_BLOCK:cb[h + 1] * N_BLOCK],
                            in_=dns[0, :, cb[h] * N_BLOCK:cb[h + 1] * N_BLOCK],
                        )
                    # The prologue put ~2.1MB on qs[0] (x + odd chunks) vs
                    # ~0.9MB on qs[1]; give qs[1] the first 4 rest-chunks,
                    # then alternate, so both queues carry ~4.5MB total and
                    # neither straggles into the last block's feed.
                    qsched = [1, 1, 1, 1]
                    rest = range(1, NUM_BLOCKS)
                else:
                    qsched = []
                    rest = range(NUM_BLOCKS)
                qi = 0
                for n in rest:
                    for h in range(rest_chunks):
                        q = qs[qsched[qi] if qi < len(qsched)
                               else (qi + len(qsched)) % 2]
                        q.dma_start(
                            out=d_sbs[n][:, h * rsize:(h + 1) * rsize],
                            in_=dns[n, :, h * rsize:(h + 1) * rsize],
                        )
                        qi += 1
                for n in range(NUM_BLOCKS):
                    d_sb = d_sbs[n]
                    last = n == NUM_BLOCKS - 1
                    if not last:
                        ps = pp.tile([BATCH, N_BLOCK], mybir.dt.float32)
                        for t in range(N_ITILES):
                            nc.tensor.matmul(
                                ps[:],
                                x_sb[:, t * 128:(t + 1) * 128],
                                d_sb[:, t * N_BLOCK:(t + 1) * N_BLOCK],
                                start=(t == 0),
                                stop=(t == N_ITILES - 1),
                            )
                        o_sb = op.tile([BATCH, N_BLOCK], mybir.dt.float16)
                        nc.vector.tensor_copy(out=o_sb[:], in_=ps[:])
                        getattr(nc, out_eng).dma_start(
                            out=out[:, n * N_BLOCK:(n + 1) * N_BLOCK],
                            in_=o_sb[:],
                        )
                        continue
                    # Last block: its drain is the exposed tail, so split
                    # into TWO accumulation groups — group A's stop lands
                    # one matmul before B's, letting A's drain overlap B's
                    # final matmul — and drain in quarters with the out-DMAs
                    # alternating both HWDGE queues so copies and transfers
                    # pipeline.
                    half = N_BLOCK // 2
                    psa = pp.tile([BATCH, half], mybir.dt.float32)
                    psb = pp.tile([BATCH, half], mybir.dt.float32)
                    for t in range(N_ITILES):
                        for g, pst in ((0, psa), (1, psb)):
                            nc.tensor.matmul(
                                pst[:],
                                x_sb[:, t * 128:(t + 1) * 128],
                                d_sb[:, t * N_BLOCK + g * half:
                                     t * N_BLOCK + (g + 1) * half],
                                start=(t == 0),
                                stop=(t == N_ITILES - 1),
                            )
                    psz = half // 2
                    for j, pst in ((0, psa), (1, psa), (2, psb), (3, psb)):
                        o_sb = op.tile([BATCH, psz], mybir.dt.float16)
                        nc.vector.tensor_copy(
                            out=o_sb[:],
                            in_=pst[:, (j % 2) * psz:(j % 2 + 1) * psz],
                        )
                        oq = qs[j % 2] if out_eng == "scalar" else getattr(
                            nc, out_eng)
                        oq.dma_start(
                            out=out[:, n * N_BLOCK + j * psz:
                                    n * N_BLOCK + (j + 1) * psz],
                            in_=o_sb[:],
                        )

    nc.compile()
    aps = {"xT": xT, "dns": dns, "out": out}
    _CACHE[key] = (nc, aps)
    return nc, aps


def _prepare_inputs(x, w, idx):
    x = np.asarray(x, dtype=np.float32)
    w = np.asarray(w, dtype=np.float32)
    idx = np.asarray(idx)

    # Scatter with last-write-wins (ascending k ⇒ later k overwrites earlier,
    # matching torch's index_put / the reference's keep-mask + scatter-add).
    dense = np.zeros((N_IN, COLS), dtype=np.float32)
    cols = np.arange(COLS)
    for k in range(N_NPB):
        dense[idx[k], cols] = w[k]
    dense = np.clip(dense * W_SCALE, -E3M4_MAX, E3M4_MAX)

    # lhsT layout: xT[il, t, b] = x[b, t*128 + il]
    xT = np.ascontiguousarray(
        x.T.reshape(N_ITILES, 128, BATCH).transpose(1, 0, 2).reshape(128, N_IN)
    ).astype(F16)

    in_maps = []
    for core in range(N_CORES):
        dc = dense[:, core * COLS_PER_CORE:(core + 1) * COLS_PER_CORE]
        # D[n, il, t, c'] = dc[t*128 + il, n*N_BLOCK + c']
        D = np.ascontiguousarray(
            dc.reshape(N_ITILES, 128, NUM_BLOCKS, N_BLOCK)
            .transpose(2, 1, 0, 3)
            .reshape(NUM_BLOCKS, 128, N_ITILES * N_BLOCK)
        ).astype(E3M4)
        in_maps.append({"xT": xT, "dns": D})
    return in_maps


def _run(in_maps, trace=False):
    nc, _ = _build_program()
    res = bass_utils.run_bass_kernel_spmd(
        nc, in_maps, core_ids=list(range(N_CORES)), trace=trace
    )
    _CACHE["last_results"] = res
    return res


def _gather(res):
    out = np.concatenate(
        [np.asarray(r["out"], dtype=np.float32) for r in res.results], axis=1
    )
    return (out / W_SCALE).reshape(BATCH, N_B, N_NEXT_H).astype(np.float32)


def kernel(x, w, idx):
    in_maps = _prepare_inputs(x, w, idx)
    out = None
    for attempt in range(3):
        try:
            out = _gather(_run(in_maps, trace=False))
        except Exception:
            # A previously wedged device can fail the first attach; a retry
            # on a fresh execution is usually enough (device resets on
            # attach).
            import time
            time.sleep(2.0)
            continue
        # A wedged device can also return garbage instead of raising —
        # outputs are bounded by ~0.6 here, so NaN/huge values mean the
        # execution itself was bad, not the math.
        if np.isfinite(out).all() and np.abs(out).max() < 100.0:
            return out
    if out is None:
        raise RuntimeError("kernel execution failed repeatedly")
    return out


# revision 35
# speedup vs baseline: 7.6811x; 1.2474x over previous
"""BranchLayer kernel for 8 Trainium2 NeuronCores.

Math: out[b, c] = sum_k x[b, idx[k, c]] * w[k, c], with last-write-wins on
duplicate (idx[k,c], c) pairs — i.e. out = x @ dense where
dense[i, c] = w[k_last, c] for the last k with idx[k, c] == i.

Strategy (sharding_hint): shard the COLS=16384 column dim of dense across the
8 cores (2048 columns each); x is replicated. The host scatters w into dense
(cheap index bookkeeping) and quantizes it to fp8-e3m4 (x stays fp16), so the
dominant HBM stream halves vs fp16 and drops below the TensorE floor of
65536 cycles (~27us @2.4GHz) — the kernel is PE-bound, so the x load is
split into pieces interleaved with the first dense chunks to start the
matmul pipeline (and the PE clock ramp) as early as possible. Output ships
fp16 (values pre-scaled by W_SCALE); the host upcasts and descales.
"""

import numpy as np
import ml_dtypes

import concourse.bass as bass
import concourse.bacc as bacc
import concourse.mybir as mybir
import concourse.tile as tile
from concourse import bass_utils

F16 = np.float16
E3M4 = ml_dtypes.float8_e3m4

# Problem shape (hardcoded per task contract).
N_IN = 4096
N_NPB = 64
N_B = 64
N_NEXT_H = 256
COLS = N_B * N_NEXT_H  # 16384
BATCH = 128
N_CORES = 8

COLS_PER_CORE = COLS // N_CORES  # 2048
N_BLOCK = 512                    # output columns per PSUM block (one bank)
NUM_BLOCKS = COLS_PER_CORE // N_BLOCK  # 4
N_ITILES = N_IN // 128           # 32 contraction tiles

W_SCALE = 64.0                   # dense pre-scale so e3m4 sees ~[-4, 4]
E3M4_MAX = 15.5
X_PIECES = 4                     # x DMA split (1024 cols each)

_CACHE = {}


def _build_program(repeats=1, dbufs=4, chunks=8, rest_chunks=8,
                   warmup=10, x_pieces=X_PIECES, out_eng="scalar"):
    """One SPMD Bass program; all 8 cores run it on different dense shards.

    repeats>1 loops the whole pipeline inside one NEFF — used only for
    repeat-delta HW timing in test.py (tunnel overhead cancels).
    dbufs: dense-tile pool slots (4 = every block's DMA in flight at start).
    chunks: dense DMA chunks for block 0 (fine ⇒ early first matmul);
    rest_chunks: chunks for blocks 1..3 (coarse — HWDGE queue config time
    per dma_start is ~0.6us, so fewer transfers once the pipe is primed).
    warmup: N=256 dummy matmuls on memset-only tiles (no DMA dependency)
    that ramp the PE clock out of its low p-states during the ~2.5us DMA
    start latency — the kernel is PE-bound, so real matmuls must hit full
    clock immediately.
    """
    if repeats > 1 and out_eng == "scalar":
        # In repeat-timing programs a rep's out-DMAs on the scalar HWDGE
        # queue would stall the NEXT rep's load chunks behind a compute
        # wait; route them via SWDGE there. (Irrelevant for the shipped
        # repeats=1 program — nothing follows its outs.)
        out_eng = "gpsimd"
    key = ("nc", repeats, dbufs, chunks, rest_chunks, warmup, x_pieces,
           out_eng)
    if key in _CACHE:
        return _CACHE[key]

    nc = bacc.Bacc(
        "TRN2",
        target_bir_lowering=False,
        debug=False,
        enable_asserts=False,
        num_devices=N_CORES,
    )
    # xT[il, t*128 + b] = x[b, t*128 + il]  (lhsT tiles, fp16)
    xT = nc.dram_tensor("xT", [128, N_IN], mybir.dt.float16, kind="ExternalInput").ap()
    # dns[n, il, t*N_BLOCK + c'] = dense[t*128 + il, n*N_BLOCK + c'] (per-core
    # shard, e3m4, pre-scaled by W_SCALE)
    dns = nc.dram_tensor(
        "dns", [NUM_BLOCKS, 128, N_ITILES * N_BLOCK], mybir.dt.float8e3,
        kind="ExternalInput",
    ).ap()
    out = nc.dram_tensor(
        "out", [BATCH, COLS_PER_CORE], mybir.dt.float16, kind="ExternalOutput"
    ).ap()

    with tile.TileContext(nc) as tc:
        with (
            tc.tile_pool(name="xp", bufs=1) as xp,
            tc.tile_pool(name="dp", bufs=dbufs) as dp,
            tc.tile_pool(name="op", bufs=6) as op,
            tc.tile_pool(name="pp", bufs=2, space="PSUM") as pp,
        ):
            x_sb = xp.tile([128, N_IN], mybir.dt.float16)
            rsize = N_ITILES * N_BLOCK // rest_chunks
            qs = [nc.sync, nc.scalar]
            # Block-0 chunk boundaries (subtile units) grow geometrically:
            # the first matmul only waits for a 2-subtile (128KB) transfer,
            # pulling the whole PE schedule ~0.6us earlier. x pieces are
            # sized so piece p always lands before the chunks that need it.
            if chunks == 8:
                cb = [0, 1, 2, 4, 8, 12, 17, 24, 32]
            else:
                cs = N_ITILES // chunks
                cb = [i * cs for i in range(chunks + 1)]
            if x_pieces == 4:
                # piece 0 covers exactly the subtiles chunk 0-1 need (64KB,
                # same transfer time as chunk 0) — x and dense arrive
                # together for the first matmul.
                xb = [0, 256, 1024, 2560, 4096]
            else:
                xs = N_IN // x_pieces
                xb = [i * xs for i in range(x_pieces + 1)]

            if warmup:
                # PE clock ramp: dummy matmuls against a RAW (non-pool)
                # SBUF tensor — no producer instruction at all, so the tile
                # scheduler sees no deps and the warms issue right after the
                # prologue barrier (~0.7us), while the first loads are still
                # in HWDGE config. Uninitialized operands (even NaN) are
                # harmless: results land in a scratch PSUM bank nobody
                # reads. Small N so the warm queue drains right as the
                # first real chunk lands.
                w_raw = nc.alloc_sbuf_tensor(
                    "warm_src", [128, 256 + 128], mybir.dt.float16
                ).ap()
                warm_ps = pp.tile([BATCH, 256], mybir.dt.float32, tag="warm")
                for _wmm in range(warmup):
                    nc.tensor.matmul(
                        warm_ps[:],
                        w_raw[:, 256:],
                        w_raw[:, :256],
                        start=True,
                        stop=True,
                    )

            for _rep in range(repeats):
                # Block tiles all live at once (dbufs=NUM_BLOCKS) so every
                # load chunk is issued before any compute-dependent
                # instruction lands on the HWDGE queues.
                d_sbs = []
                for _n in range(NUM_BLOCKS):
                    d_sb = dp.tile([128, N_ITILES * N_BLOCK], mybir.dt.float8e3)
                    d_sbs.append(d_sb)
                # The kernel is PE-bound: interleave x pieces with block 0's
                # chunks across the two HWDGE queues so matmul t=0 (needs
                # x piece 0 + d0 chunk 0 only — tile deps are slice-level)
                # starts as soon as the first two transfers land.
                if _rep == 0:
                    # every x piece must fit in the even-h slots below
                    assert chunks >= 2 * x_pieces
                    for h in range(chunks):
                        if h % 2 == 0 and h // 2 < x_pieces:
                            p = h // 2
                            qs[0].dma_start(
                                out=x_sb[:, xb[p]:xb[p + 1]],
                                in_=xT[:, xb[p]:xb[p + 1]],
                            )
                        qs[1 if h % 2 == 0 else 0].dma_start(
                            out=d_sbs[0][:, cb[h] * N_BLOCK:cb[h + 1] * N_BLOCK],
                            in_=dns[0, :, cb[h] * N_BLOCK:cb[h + 1] * N_BLOCK],
                        )
                    # The prologue put ~2.1MB on qs[0] (x + odd chunks) vs
                    # ~0.9MB on qs[1]; give qs[1] the first 4 rest-chunks,
                    # then alternate, so both queues carry ~4.5MB total and
                    # neither straggles into the last block's feed.
                    qsched = [1, 1, 1, 1]
                    rest = range(1, NUM_BLOCKS)
                else:
                    qsched = []
                    rest = range(NUM_BLOCKS)
                qi = 0
                for n in rest:
                    for h in range(rest_chunks):
                        q = qs[qsched[qi] if qi < len(qsched)
                               else (qi + len(qsched)) % 2]
                        q.dma_start(
                            out=d_sbs[n][:, h * rsize:(h + 1) * rsize],
                            in_=dns[n, :, h * rsize:(h + 1) * rsize],
                        )
                        qi += 1
                for n in range(NUM_BLOCKS):
                    d_sb = d_sbs[n]
                    last = n == NUM_BLOCKS - 1
                    if not last:
                        ps = pp.tile([BATCH, N_BLOCK], mybir.dt.float32)
                        for t in range(N_ITILES):
                            nc.tensor.matmul(
                                ps[:],
                                x_sb[:, t * 128:(t + 1) * 128],
                                d_sb[:, t * N_BLOCK:(t + 1) * N_BLOCK],
                                start=(t == 0),
                                stop=(t == N_ITILES - 1),
                            )
                        o_sb = op.tile([BATCH, N_BLOCK], mybir.dt.float16)
                        nc.vector.tensor_copy(out=o_sb[:], in_=ps[:])
                        getattr(nc, out_eng).dma_start(
                            out=out[:, n * N_BLOCK:(n + 1) * N_BLOCK],
                            in_=o_sb[:],
                        )
                        continue
                    # Last block: its drain is the exposed tail, so split
                    # into TWO accumulation groups — group A's stop lands
                    # one matmul before B's, letting A's drain overlap B's
                    # final matmul — and drain in quarters with the out-DMAs
                    # alternating both HWDGE queues so copies and transfers
                    # pipeline.
                    # Own single-slot rings (fresh banks): if these shared
                    # the ps ring, psb would reuse block 2's slot and stall
                    # ~0.4us on its drain copy at the block 2->3 boundary.
                    half = N_BLOCK // 2
                    psa = pp.tile([BATCH, half], mybir.dt.float32, bufs=1)
                    psb = pp.tile([BATCH, half], mybir.dt.float32, bufs=1)
                    for t in range(N_ITILES):
                        for g, pst in ((0, psa), (1, psb)):
                            nc.tensor.matmul(
                                pst[:],
                                x_sb[:, t * 128:(t + 1) * 128],
                                d_sb[:, t * N_BLOCK + g * half:
                                     t * N_BLOCK + (g + 1) * half],
                                start=(t == 0),
                                stop=(t == N_ITILES - 1),
                            )
                    psz = half // 2
                    for j, pst in ((0, psa), (1, psa), (2, psb), (3, psb)):
                        o_sb = op.tile([BATCH, psz], mybir.dt.float16)
                        nc.vector.tensor_copy(
                            out=o_sb[:],
                            in_=pst[:, (j % 2) * psz:(j % 2 + 1) * psz],
                        )
                        oq = qs[j % 2] if out_eng == "scalar" else getattr(
                            nc, out_eng)
                        oq.dma_start(
                            out=out[:, n * N_BLOCK + j * psz:
                                    n * N_BLOCK + (j + 1) * psz],
                            in_=o_sb[:],
                        )

    nc.compile()
    aps = {"xT": xT, "dns": dns, "out": out}
    _CACHE[key] = (nc, aps)
    return nc, aps


def _prepare_inputs(x, w, idx):
    x = np.asarray(x, dtype=np.float32)
    w = np.asarray(w, dtype=np.float32)
    idx = np.asarray(idx)

    # Scatter with last-write-wins (ascending k ⇒ later k overwrites earlier,
    # matching torch's index_put / the reference's keep-mask + scatter-add).
    dense = np.zeros((N_IN, COLS), dtype=np.float32)
    cols = np.arange(COLS)
    for k in range(N_NPB):
        dense[idx[k], cols] = w[k]
    dense = np.clip(dense * W_SCALE, -E3M4_MAX, E3M4_MAX)

    # lhsT layout: xT[il, t, b] = x[b, t*128 + il]
    xT = np.ascontiguousarray(
        x.T.reshape(N_ITILES, 128, BATCH).transpose(1, 0, 2).reshape(128, N_IN)
    ).astype(F16)

    in_maps = []
    for core in range(N_CORES):
        dc = dense[:, core * COLS_PER_CORE:(core + 1) * COLS_PER_CORE]
        # D[n, il, t, c'] = dc[t*128 + il, n*N_BLOCK + c']
        D = np.ascontiguousarray(
            dc.reshape(N_ITILES, 128, NUM_BLOCKS, N_BLOCK)
            .transpose(2, 1, 0, 3)
            .reshape(NUM_BLOCKS, 128, N_ITILES * N_BLOCK)
        ).astype(E3M4)
        in_maps.append({"xT": xT, "dns": D})
    return in_maps


def _run(in_maps, trace=False):
    nc, _ = _build_program()
    res = bass_utils.run_bass_kernel_spmd(
        nc, in_maps, core_ids=list(range(N_CORES)), trace=trace
    )
    _CACHE["last_results"] = res
    return res


def _gather(res):
    out = np.concatenate(
        [np.asarray(r["out"], dtype=np.float32) for r in res.results], axis=1
    )
    return (out / W_SCALE).reshape(BATCH, N_B, N_NEXT_H).astype(np.float32)


def kernel(x, w, idx):
    in_maps = _prepare_inputs(x, w, idx)
    out = None
    for attempt in range(3):
        try:
            out = _gather(_run(in_maps, trace=False))
        except Exception:
            # A previously wedged device can fail the first attach; a retry
            # on a fresh execution is usually enough (device resets on
            # attach).
            import time
            time.sleep(2.0)
            continue
        # A wedged device can also return garbage instead of raising —
        # outputs are bounded by ~0.6 here, so NaN/huge values mean the
        # execution itself was bad, not the math.
        if np.isfinite(out).all() and np.abs(out).max() < 100.0:
            return out
    if out is None:
        raise RuntimeError("kernel execution failed repeatedly")
    return out
